# revision 32
# baseline (speedup 1.0000x reference)
"""AttentionBlock (GroupNorm -> 1x1 qkv -> single-head attention over 64x64
spatial -> out proj -> residual) on 8 TRN2 NeuronCores via Bass/Tile.

Sharding: core = (sample s = core//2) x (query-half h = core%2). Each core
computes K/V (via folded weights) for its full sample and attention rows for
its half of the 4096 spatial positions.

Host-side input prep (exact algebra, no device cost):
  - each core's x columns are rotated so its query half sits at columns
    0:2048 (attention is permutation-invariant over key order);
  - scores^T[j,i] = k_j . q_i = xn_j^T (Wk^T Wq) xn_i + (Wk^T bq) . xn_j
    + const(i): qq = (Wk^T Wq) xn_q + u with u = Wk^T bq, and const(i)
    drops under softmax, so q and k are never formed;
  - Wo and Wv fold into W2 = Wo @ Wv (applied after the attention-weighted
    sum of xn, which is produced via PE transposes of xn instead of a V
    projection); v-bias and out-bias fold into fbias = out_b + out_w @ bv.
Softmax runs without max-subtraction (scores are ~N(0,1) here). All matmuls
are bf16 with fp32 PSUM accumulation; softmax denominators accumulate
exp tiles via a small DVE add-tree + ones-matmul. Validated ~3e-4
scale-relative absmax error vs the fp32 reference.
"""

from contextlib import ExitStack

import numpy as np
import ml_dtypes

import concourse.bacc as bacc
import concourse.tile as tile
from concourse import mybir
from concourse.bass_utils import run_bass_kernel_spmd

N_CORES = 8
B = 4                 # batch
C = 512               # channels
HW = 4096             # 64*64 spatial positions
P = 128               # partitions
NT = C // P           # 4 channel tiles
NJ = HW // P          # 32 key tiles
QC = HW // 2          # 2048 query columns per core
NIT = QC // 512       # 4 query i-tiles per core
G = 32                # groupnorm groups
EPS = 1e-5
SCALE = float(C) ** -0.5

F32 = mybir.dt.float32
BF16 = mybir.dt.bfloat16

_CACHED_NC = None

# build-time feature flags (ablation/testing only; defaults are production)
CFG = {
    "attention": True,   # emit the attention i-loop
    "qkv": True,         # emit qq/vT projections
    "gn_stats": True,
    "gn_combine": True,
    "gn_apply": True,
}


def _emit(ctx: ExitStack, tc: tile.TileContext, xs, xh, wa, wz, ident, uu, gw, gb, fb, yo):
    nc = tc.nc
    mult = mybir.AluOpType.mult
    add = mybir.AluOpType.add

    singles = ctx.enter_context(tc.tile_pool(name="singles", bufs=1))
    wpool = ctx.enter_context(tc.tile_pool(name="wpool", bufs=1))
    gn = ctx.enter_context(tc.tile_pool(name="gn", bufs=1))
    st_pool = ctx.enter_context(tc.tile_pool(name="st", bufs=2))
    e_pool = ctx.enter_context(tc.tile_pool(name="e", bufs=6))
    rec_pool = ctx.enter_context(tc.tile_pool(name="rec", bufs=2))
    hn_pool = ctx.enter_context(tc.tile_pool(name="hn", bufs=2))
    osb_pool = ctx.enter_context(tc.tile_pool(name="osb", bufs=2))
    ps_s = ctx.enter_context(tc.tile_pool(name="ps_s", bufs=3, space="PSUM"))
    ps_acc = ctx.enter_context(tc.tile_pool(name="ps_acc", bufs=4, space="PSUM"))
    ps_dn = ctx.enter_context(tc.tile_pool(name="ps_dn", bufs=1, space="PSUM"))

    # ---- constants / weights in SBUF ----
    ones_bf = wpool.tile([P, P], BF16)
    nc.vector.memset(ones_bf[:], 1.0)
    eps_t = wpool.tile([P, 1], F32)
    nc.vector.memset(eps_t[:], EPS)

    # ---- load x (full sample, per channel-tile) + stats ----
    xs_r = xs.rearrange("(t p) n -> t p n", p=P)
    xh_r = xh.rearrange("(t p) n -> t p n", p=P)
    # stats32[p, 2t] = mean of channel 128t+p; [p, 2t+1] = var (then E[x^2])
    stats32 = gn.tile([P, 32], F32, tag="s32")
    nc.vector.memset(stats32[:], 0.0)
    xf = []   # fp32 query half
    xhb = []  # bf16 other half
    for t in range(NT):
        xf_t = singles.tile([P, QC], F32, tag=f"xf{t}", name=f"xf{t}")
        nc.sync.dma_start(out=xf_t[:], in_=xs_r[t])
        xf.append(xf_t[:])
        xh_t = singles.tile([P, QC], BF16, tag=f"xh{t}", name=f"xh{t}")
        # finer pieces for the last tile so the stats chain starts during
        # the transfer tail
        nch = 4 if t == NT - 1 else 2
        for c4 in range(nch):
            w = QC // nch
            nc.sync.dma_start(out=xh_t[:, c4 * w:(c4 + 1) * w],
                              in_=xh_r[t][:, c4 * w:(c4 + 1) * w])
        xhb.append(xh_t[:])
        st6 = st_pool.tile([P, 8, 6], F32, tag="st6")
        for s8 in range(8 if CFG["gn_stats"] else 0):
            half = xf[t] if s8 < 4 else xhb[t]
            off = s8 * 512 if s8 < 4 else (s8 - 4) * 512
            nc.vector.bn_stats(out=st6[:, s8, :], in_=half[:, off:off + 512])
        if CFG["gn_stats"]:
            nc.vector.bn_aggr(out=stats32[:, 2 * t:2 * t + 2], in_=st6[:])

    wa_sb = wpool.tile([P, NT, C], BF16)
    nc.sync.dma_start(out=wa_sb[:], in_=wa.rearrange("(t p) n -> p t n", p=P))
    wz_sb = wpool.tile([P, NT, C], BF16)
    nc.sync.dma_start(out=wz_sb[:], in_=wz.rearrange("(t p) n -> p t n", p=P))
    id_sb = wpool.tile([P, P], BF16)
    nc.sync.dma_start(out=id_sb[:], in_=ident)
    u_sb = wpool.tile([P, NT], F32)
    nc.sync.dma_start(out=u_sb[:], in_=uu.rearrange("(t p) -> p t", p=P))
    gw_sb = wpool.tile([P, NT], F32)
    nc.scalar.dma_start(out=gw_sb[:], in_=gw.rearrange("(t p) -> p t", p=P))
    gb_sb = wpool.tile([P, NT], F32)
    nc.scalar.dma_start(out=gb_sb[:], in_=gb.rearrange("(t p) -> p t", p=P))
    fb_sb = wpool.tile([P, NT], F32)
    nc.sync.dma_start(out=fb_sb[:], in_=fb.rearrange("(t p) -> p t", p=P))

    # ---- group stats via 32x32 block transposes ----
    # stats32[p, 2t]   = mean of channel 128t+p
    # stats32[p, 2t+1] = E[x^2] of channel 128t+p
    if CFG["gn_combine"]:
        for t in range(NT):
            # var -> E[x^2]: odd col += mean^2
            nc.vector.scalar_tensor_tensor(
                out=stats32[:, 2 * t + 1:2 * t + 2],
                in0=stats32[:, 2 * t:2 * t + 1], scalar=stats32[:, 2 * t:2 * t + 1],
                in1=stats32[:, 2 * t + 1:2 * t + 2],
                op0=mult, op1=add,
            )
        t32 = gn.tile([P, 32], F32, tag="t32")
        nc.vector.transpose(out=t32[:], in_=stats32[:])
        gsum = gn.tile([P, 2], F32, tag="gsum")
        nc.vector.tensor_reduce(
            out=gsum[:], in_=t32[:].rearrange("p (g r) -> p g r", g=2),
            axis=mybir.AxisListType.X, op=add,
        )
        b32 = gn.tile([P, 2, 16], F32, tag="b32")
        nc.vector.tensor_scalar(
            out=b32[:], in0=gsum[:].to_broadcast([P, 2, 16]),
            scalar1=1.0 / 16.0, scalar2=None, op0=mult)
        tb = gn.tile([P, 32], F32, tag="tb")
        nc.vector.transpose(out=tb[:], in_=b32[:].rearrange("p g r -> p (g r)"))

        # per-partition group stats, one column per channel-tile
        tbv = tb[:].rearrange("p (t s) -> p t s", s=2)
        m_g = tbv[:, 0:NT, 0]
        var_g = gn.tile([P, NT], F32, tag="varg")
        nc.vector.scalar_tensor_tensor(
            out=var_g[:], in0=m_g, scalar=-1.0, in1=m_g,
            op0=mult, op1=mult)
        nc.vector.tensor_add(out=var_g[:], in0=tbv[:, 0:NT, 1], in1=var_g[:])
        # rstd = exp(-0.5 * ln(var + eps)) -- stays in the natural_log_exp table set
        rstd = gn.tile([P, NT], F32, tag="rstd")
        nc.scalar.activation(out=rstd[:], in_=var_g[:],
                             func=mybir.ActivationFunctionType.Ln, bias=eps_t[:], scale=1.0)
        nc.scalar.activation(out=rstd[:], in_=rstd[:],
                             func=mybir.ActivationFunctionType.Exp, scale=-0.5)
        a_sc = gn.tile([P, NT], F32, tag="asc")
        nc.vector.tensor_mul(out=a_sc[:], in0=rstd[:], in1=gw_sb[:])
        b_sc = gn.tile([P, NT], F32, tag="bsc")
        nc.vector.tensor_mul(out=b_sc[:], in0=m_g, in1=a_sc[:])
        nc.vector.tensor_sub(out=b_sc[:], in0=gb_sb[:], in1=b_sc[:])

    # ---- apply GN: xn = a*x + b ----
    xnbf = []
    for t in range(NT):
        xnbf_t = singles.tile([P, HW], BF16, tag=f"xnbf{t}", name=f"xnbf{t}")
        xnbf.append(xnbf_t)
    if CFG["gn_apply"]:
        # chunked so the first query-column chunks land first and PE can start;
        # split across DVE and ACT
        for ch in range(8):
            sl = slice(ch * 512, (ch + 1) * 512)
            src_half = xf if ch < 4 else xhb
            sl_in = sl if ch < 4 else slice((ch - 4) * 512, (ch - 3) * 512)
            for t in range(NT):
                nc.vector.tensor_scalar(
                    out=xnbf[t][:, sl], in0=src_half[t][:, sl_in],
                    scalar1=a_sc[:, t:t + 1], scalar2=b_sc[:, t:t + 1],
                    op0=mult, op1=add,
                )
        for t in range(NT):
            nc.gpsimd.tensor_scalar(
                out=xf[t][:, 0:QC], in0=xf[t][:, 0:QC],
                scalar1=a_sc[:, t:t + 1], scalar2=b_sc[:, t:t + 1], op0=mult, op1=add,
            )

    # ---- qq = A @ xn_q + u   (bf16, [co, i] layout) ----
    qq_sb = singles.tile([P, NT, QC], BF16, tag="qq")
    for mo in range(NT if CFG["qkv"] else 0):
        for itc in range(NIT):
            ps = ps_s.tile([P, 512], F32, tag="mm")
            for kt in range(NT):
                nc.tensor.matmul(
                    ps[:], wa_sb[:, kt, mo * P:(mo + 1) * P],
                    xnbf[kt][:, itc * 512:(itc + 1) * 512],
                    start=(kt == 0), stop=(kt == NT - 1),
                )
            nc.scalar.add(out=qq_sb[:, mo, itc * 512:(itc + 1) * 512], in_=ps[:],
                          add=u_sb[:, mo:mo + 1])

    # ---- xn^T via PE transpose ([j, c] layout; replaces V: W2 folds Wo@Wv) ----
    vt = []
    for half in range(2):
        vt.append(singles.tile([P, NJ // 2, C], BF16, tag=f"vt{half}", name=f"vt{half}"))
    for jt in range(NJ if CFG["qkv"] else 0):
        ps = ps_acc.tile([P, 4, P], BF16, tag="acc", name=f"trps{jt}")
        for kt in range(NT):
            nc.tensor.transpose(
                out=ps[:, kt, :], in_=xnbf[kt][:, jt * P:(jt + 1) * P],
                identity=id_sb[:],
            )
        if jt % 2 == 0:
            nc.vector.tensor_copy(out=vt[jt // 16][:, jt % 16, :], in_=ps[:])
        else:
            nc.scalar.copy(out=vt[jt // 16][:, jt % 16, :], in_=ps[:])

    # ---- attention over i-tiles ----
    yo_r = yo.rearrange("(t p) n -> p t n", p=P)
    if not CFG["attention"]:
        out_sb0 = osb_pool.tile([P, NT, 512], F32, tag="osb")
        nc.vector.tensor_copy(out=out_sb0[:], in_=xf[0][:, 0:2048].rearrange('p (a b) -> p a b', a=NT))
        nc.sync.dma_start(out=yo_r[:, :, 0:512], in_=out_sb0[:])
    for it in range(NIT if CFG["attention"] else 0):
        h_ps = [ps_acc.tile([P, 512], F32, tag="acc", name=f"hps{it}_{ct}") for ct in range(NT)]
        dn_ps = ps_dn.tile([P, 512], F32, tag="dn")
        def emit_av(jt, e_sb):
            for ct in range(NT):
                nc.tensor.matmul(
                    h_ps[ct][:], vt[jt // 16][:, jt % 16, ct * P:(ct + 1) * P], e_sb[:],
                    start=(jt == 0), stop=(jt == NJ - 1),
                )

        # software pipeline: scores(jt+2) issue before AV(jt) so the exp
        # latency never stalls the PE stream
        pending_av = []
        pending_dn = None
        for jt in range(NJ):
            s_ps = ps_s.tile([P, 512], F32, tag="mm")
            for kt in range(NT):
                nc.tensor.matmul(
                    s_ps[:], xnbf[kt][:, jt * P:(jt + 1) * P],
                    qq_sb[:, kt, it * 512:(it + 1) * 512],
                    start=(kt == 0), stop=(kt == NT - 1),
                )
            e_sb = e_pool.tile([P, 512], BF16, tag="e")
            nc.scalar.activation(out=e_sb[:], in_=s_ps[:],
                                 func=mybir.ActivationFunctionType.Exp, scale=SCALE)
            if pending_dn is not None:
                dn_jt, dn_e = pending_dn
                nc.tensor.matmul(dn_ps[:], ones_bf[:], dn_e[:],
                                 start=(dn_jt == 7), stop=(dn_jt == NJ - 1))
                pending_dn = None
            if jt % 2 == 0:
                e_prev = e_sb
            else:
                e_pair = e_pool.tile([P, 512], BF16, tag="epair", bufs=2, name=f"epair{it}_{jt}")
                nc.vector.tensor_add(out=e_pair[:], in0=e_prev[:], in1=e_sb[:])
                if jt % 4 == 1:
                    e_pair_prev = e_pair
                else:
                    e_quad = e_pool.tile([P, 512], BF16, tag="equad", bufs=2, name=f"equad{it}_{jt}")
                    nc.vector.tensor_add(out=e_quad[:], in0=e_pair_prev[:], in1=e_pair[:])
                    if jt % 8 == 3:
                        e_quad_prev = e_quad
                    else:
                        e_oct = e_pool.tile([P, 512], BF16, tag="eoct", bufs=2, name=f"eoct{it}_{jt}")
                        nc.vector.tensor_add(out=e_oct[:], in0=e_quad_prev[:], in1=e_quad[:])
                        pending_dn = (jt, e_oct)
            pending_av.append((jt, e_sb))
            if len(pending_av) > 2:
                emit_av(*pending_av.pop(0))
        for args in pending_av:
            emit_av(*args)
        assert pending_dn is not None
        dn_jt, dn_e = pending_dn
        nc.tensor.matmul(dn_ps[:], ones_bf[:], dn_e[:],
                         start=(dn_jt == 7), stop=(dn_jt == NJ - 1))
        pending_dn = None
        rec = rec_pool.tile([P, 512], F32, tag="rec")
        nc.vector.reciprocal(out=rec[:], in_=dn_ps[:])
        hn = hn_pool.tile([P, NT, 512], BF16, tag="hn")
        for ct in range(NT):
            nc.vector.tensor_mul(out=hn[:, ct, :], in0=h_ps[ct][:], in1=rec[:])
        out_sb = osb_pool.tile([P, NT, 512], F32, tag="osb")
        for mo in range(NT):
            o_ps = ps_acc.tile([P, 512], F32, tag="acc")
            for ct in range(NT):
                nc.tensor.matmul(
                    o_ps[:], wz_sb[:, ct, mo * P:(mo + 1) * P], hn[:, ct, :],
                    start=(ct == 0), stop=(ct == NT - 1),
                )
            nc.vector.scalar_tensor_tensor(
                out=out_sb[:, mo, :], in0=o_ps[:], scalar=fb_sb[:, mo:mo + 1],
                in1=xf[mo][:, it * 512:(it + 1) * 512], op0=add, op1=add,
            )
            nc.sync.dma_start(out=yo_r[:, mo, it * 512:(it + 1) * 512],
                              in_=out_sb[:, mo, :])


def _build():
    nc = bacc.Bacc("TRN2", target_bir_lowering=False, debug=False,
                   num_devices=N_CORES)
    xs = nc.dram_tensor("xs", [C, QC], F32, kind="ExternalInput").ap()
    xh = nc.dram_tensor("xh", [C, QC], BF16, kind="ExternalInput").ap()
    wa = nc.dram_tensor("wa", [C, C], BF16, kind="ExternalInput").ap()
    wz = nc.dram_tensor("wz", [C, C], BF16, kind="ExternalInput").ap()
    ident = nc.dram_tensor("ident", [P, P], BF16, kind="ExternalInput").ap()
    uu = nc.dram_tensor("uu", [C], F32, kind="ExternalInput").ap()
    gw = nc.dram_tensor("gw", [C], F32, kind="ExternalInput").ap()
    gb = nc.dram_tensor("gb", [C], F32, kind="ExternalInput").ap()
    fb = nc.dram_tensor("fb", [C], F32, kind="ExternalInput").ap()
    yo = nc.dram_tensor("yo", [C, QC], F32, kind="ExternalOutput").ap()
    with tile.TileContext(nc) as tc:
        with ExitStack() as ctx:
            _emit(ctx, tc, xs, xh, wa, wz, ident, uu, gw, gb, fb, yo)
    nc.compile()
    return nc


def get_program():
    global _CACHED_NC
    if _CACHED_NC is None:
        _CACHED_NC = _build()
    return _CACHED_NC


def make_in_maps(x, gn_weight, gn_bias, qkv_w, qkv_b, out_w, out_b):
    x = np.ascontiguousarray(np.asarray(x, dtype=np.float32))
    qkv_w = np.asarray(qkv_w, dtype=np.float32)
    qkv_b = np.asarray(qkv_b, dtype=np.float32)
    out_w = np.asarray(out_w, dtype=np.float32)
    out_b = np.asarray(out_b, dtype=np.float32)

    wq_h, wk_h, wv_h = qkv_w[:C], qkv_w[C:2 * C], qkv_w[2 * C:]
    bq, bv = qkv_b[:C], qkv_b[2 * C:]
    wa_h = (wq_h.T @ wk_h).astype(ml_dtypes.bfloat16)      # (Wk^T Wq)^T = [c_in, c_out]
    u_h = (wk_h.T @ bq).astype(np.float32)
    wz_t = np.ascontiguousarray((out_w @ wv_h).T).astype(ml_dtypes.bfloat16)
    ident_h = np.eye(P, dtype=ml_dtypes.bfloat16)
    fb_h = (out_b + out_w @ bv).astype(np.float32)
    gw_h = np.asarray(gn_weight, dtype=np.float32)
    gb_h = np.asarray(gn_bias, dtype=np.float32)

    xs_all = x.reshape(B, C, HW)
    in_maps = []
    for core in range(N_CORES):
        s, h = core // 2, core % 2
        # query half fp32; other half bf16 (only feeds stats + bf16 xn).
        # Query half sits at columns 0:QC -- attention is permutation-
        # invariant over key order, stats over all columns.
        xs_c = np.ascontiguousarray(xs_all[s][:, h * QC:(h + 1) * QC])
        xh_c = np.ascontiguousarray(
            xs_all[s][:, (1 - h) * QC:(2 - h) * QC]).astype(ml_dtypes.bfloat16)
        in_maps.append({
            "xs": xs_c, "xh": xh_c,
            "wa": wa_h, "wz": wz_t, "ident": ident_h,
            "uu": u_h, "gw": gw_h, "gb": gb_h, "fb": fb_h,
        })
    return in_maps


def _kernel_direct(x, gn_weight, gn_bias, qkv_w, qkv_b, out_w, out_b):
    nc = get_program()
    in_maps = make_in_maps(x, gn_weight, gn_bias, qkv_w, qkv_b, out_w, out_b)
    res = run_bass_kernel_spmd(nc, in_maps, list(range(N_CORES)))
    out = np.empty((B, C, HW), dtype=np.float32)
    for core in range(N_CORES):
        s, h = core // 2, core % 2
        out[s][:, h * QC:(h + 1) * QC] = res.results[core]["yo"]
    return out.reshape(B, C, 64, 64)


def _neuron_backend_available():
    """The SPMD run needs the 8 axon-tunneled NeuronCores. If the calling
    process pinned jax to cpu (JAX_PLATFORMS=cpu is common for running the
    reference), the axon backend never registers and we must re-exec."""
    try:
        import jax
        return any(d.platform not in ("cpu",) for d in jax.devices())
    except Exception:  # noqa: BLE001
        return False


def _kernel_subprocess(**inputs):
    import os
    import subprocess
    import sys
    import tempfile

    d = tempfile.mkdtemp(prefix="bass_attn_")
    np.savez(os.path.join(d, "in.npz"), **inputs)
    env = dict(os.environ)
    env.pop("JAX_PLATFORMS", None)
    here = os.path.dirname(os.path.abspath(__file__))
    script = (
        "import sys, numpy as np\n"
        f"sys.path.insert(0, {here!r})\n"
        "import kernel\n"
        f"z = np.load({os.path.join(d, 'in.npz')!r})\n"
        "out = kernel._kernel_direct(**{k: z[k] for k in z.files})\n"
        f"np.save({os.path.join(d, 'out.npy')!r}, out)\n"
    )
    subprocess.run([sys.executable, "-c", script], env=env, check=True)
    return np.load(os.path.join(d, "out.npy"))


def kernel(x, gn_weight, gn_bias, qkv_w, qkv_b, out_w, out_b):
    import time

    last_err = None
    if _neuron_backend_available():
        for delay in (3.0, 3.0):
            try:
                return _kernel_direct(x, gn_weight, gn_bias, qkv_w, qkv_b,
                                      out_w, out_b)
            except Exception as e:  # noqa: BLE001 -- transient device wedges
                last_err = e
                time.sleep(delay)
    # cpu-pinned caller, or the device claim is wedged: a fresh process gets
    # a fresh claim; wedges can last ~a minute, so back off progressively
    for delay in (5.0, 15.0, 30.0):
        try:
            return _kernel_subprocess(x=x, gn_weight=gn_weight, gn_bias=gn_bias,
                                      qkv_w=qkv_w, qkv_b=qkv_b, out_w=out_w,
                                      out_b=out_b)
        except Exception as e:  # noqa: BLE001
            last_err = e
            time.sleep(delay)
    raise last_err

